# revision 8
# baseline (speedup 1.0000x reference)
"""DiffusionStep (Chebyshev K=4) GNN message passing on 8 Trainium2 cores.

Sharding: 1D dst-shard (6250 dst/core, padded 6272) with a src-half two-pass
per propagate so dma_gather's int16 indices stay < 25088. Aggregation is
rank-major: blocks of 128 dst (sorted by per-pass in-degree) are packed into
"groups" (<= 64 chunks of 128 slots). Per group: dma_gather calls write
h[src] rows straight into a staging slab, one tensor_tensor multiply applies
edge weights (step-0 broadcast of w), and one strided tensor_reduce per block
sums ranks. The hi-pass accumulator (its own degree order) is re-permuted via
dma_scatter_add through a zero-donated DRAM bounce. Chebyshev runs on SBUF
slabs; T_k slices are AllGathered into a shared DRAM table for the next
propagate. All permutations are composed into static host-built tables.
"""

import numpy as np

N = 50000
D = 64
NCORES = 8
SLICE = 6250
SLICE_PAD = 6272            # 49 * 128
NBLK = SLICE_PAD // 128     # 49
HALF = 25088                # rows per gather table (= 4 * 6272)
G = 8                       # chunks per dma_gather call (1024 idx)
GCH = 64                    # staging capacity in chunks per group

_cache = {}


def _build_structure(edge_index, edge_weight):
    src = edge_index[0].astype(np.int64)
    dst = edge_index[1].astype(np.int64)
    w = edge_weight.astype(np.float32)

    node_core = np.minimum(np.arange(N) // SLICE, NCORES - 1)
    node_local = np.arange(N) - node_core * SLICE
    src_half = (np.minimum(src // SLICE, NCORES - 1) >= 4).astype(np.int64)
    dcore = np.minimum(dst // SLICE, NCORES - 1)

    deg = np.zeros((NCORES, 2, SLICE_PAD), np.int64)
    for c in range(NCORES):
        m = dcore == c
        dl = dst[m] - c * SLICE
        s = src_half[m]
        for p in range(2):
            deg[c, p] = np.bincount(dl[s == p], minlength=SLICE_PAD)

    pi = np.zeros((NCORES, 2, SLICE_PAD), np.int64)
    rank = np.zeros((NCORES, 2, SLICE_PAD), np.int64)
    for c in range(NCORES):
        for p in range(2):
            order = np.argsort(deg[c, p], kind="stable")
            pi[c, p] = order
            rank[c, p, order] = np.arange(SLICE_PAD)

    R = np.zeros((2, NBLK), np.int64)
    for p in range(2):
        for b in range(NBLK):
            mx = 1
            for c in range(NCORES):
                dloc = pi[c, p, b * 128:(b + 1) * 128]
                mx = max(mx, int(deg[c, p, dloc].max()))
            R[p, b] = mx

    gid = node_core * SLICE_PAD + rank[node_core, 0, node_local]

    # group packing per pass: consecutive blocks, sum R <= GCH, padded to G
    groups = []      # list of (pass, chunk_base, nch_padded, [(b, off_in_group)])
    chunk_base = 0
    blk_chunk = np.zeros((2, NBLK), np.int64)   # global chunk pos of block start
    for p in range(2):
        cur = []
        cur_n = 0
        for b in range(NBLK):
            rb = int(R[p, b])
            if cur and cur_n + rb > GCH:
                n_pad = (cur_n + G - 1) // G * G
                groups.append((p, chunk_base, n_pad, cur))
                chunk_base += n_pad
                cur, cur_n = [], 0
            cur.append((b, cur_n))
            blk_chunk[p, b] = chunk_base + cur_n   # provisional (fixed below)
            cur_n += rb
        if cur:
            n_pad = (cur_n + G - 1) // G * G
            groups.append((p, chunk_base, n_pad, cur))
            chunk_base += n_pad
    TOT = chunk_base
    # fix blk_chunk: recompute with final bases
    for (p, base, n_pad, blist) in groups:
        for (b, off) in blist:
            blk_chunk[p, b] = base + off

    idx_all = np.zeros((NCORES, TOT * 128), np.int64)
    w_all = np.zeros((NCORES, 128, TOT), np.float32)

    for c in range(NCORES):
        e_idx = np.nonzero(dcore == c)[0]
        s = src_half[e_idx]
        for p in range(2):
            sel = e_idx[s == p]
            d_loc = dst[sel] - c * SLICE
            r_of_dst = rank[c, p, d_loc]
            order = np.argsort(r_of_dst, kind="stable")
            sel = sel[order]
            d_rank = r_of_dst[order]
            counts = np.bincount(d_rank, minlength=SLICE_PAD)
            starts = np.concatenate([[0], np.cumsum(counts)[:-1]])
            within = np.arange(len(sel)) - starts[d_rank]
            b_of = d_rank // 128
            chunk = blk_chunk[p, b_of] + within
            slot = chunk * 128 + (d_rank % 128)
            g = gid[src[sel]]
            idx_all[c, slot] = g - (0 if p == 0 else HALF)
            w_all[c, slot % 128, slot // 128] = w[sel]

    ncalls = TOT // G
    idx_wrapped = np.zeros((NCORES, 16, ncalls * G * 8), np.int16)
    for c in range(NCORES):
        a = idx_all[c].reshape(ncalls, G * 128)
        for call in range(ncalls):
            wrp = a[call].reshape(-1, 16).T.astype(np.uint16).view(np.int16)
            idx_wrapped[c, :, call * G * 8:(call + 1) * G * 8] = wrp

    scat_wrapped = np.zeros((NCORES, 16, SLICE_PAD // 16), np.int16)
    for c in range(NCORES):
        sc = rank[c, 0, pi[c, 1]]
        scat_wrapped[c] = sc.reshape(-1, 16).T.astype(np.uint16).view(np.int16)

    meta = dict(R=R, TOT=TOT, ncalls=ncalls, groups=groups, pi=pi,
                rank=rank, gid=gid, node_core=node_core)
    return meta, idx_wrapped, w_all, scat_wrapped


def _build_program(meta, timing_rep=0):
    import concourse.bacc as bacc
    import concourse.mybir as mybir
    import concourse.tile as tile

    R = meta["R"]
    TOT = meta["TOT"]
    ncalls = meta["ncalls"]
    groups = meta["groups"]

    nc = bacc.Bacc(num_devices=NCORES, num_swdge_queues=2)
    f32 = mybir.dt.float32

    x_sl = nc.dram_tensor("x_sl", [SLICE_PAD, D], f32, kind="ExternalInput")
    x_bounce = nc.dram_tensor("x_bounce", [SLICE_PAD, D], f32, kind="Internal")
    x_glob = nc.dram_tensor("x_glob", [2 * HALF, D], f32, kind="Internal",
                            addr_space="Shared")
    idx_in = nc.dram_tensor("idx", [16, ncalls * G * 8], mybir.dt.int16,
                            kind="ExternalInput")
    w_in = nc.dram_tensor("w", [128, TOT], f32, kind="ExternalInput")
    scat_in = nc.dram_tensor("scat", [16, SLICE_PAD // 16], mybir.dt.int16,
                             kind="ExternalInput")
    out_t = nc.dram_tensor("out", [4, SLICE_PAD, D], f32, kind="ExternalOutput")
    bounce = [nc.dram_tensor(f"bnc{k}", [SLICE_PAD, D], f32,
                             kind="Internal") for k in range(4)]
    ag_in = [nc.dram_tensor(f"agin{k}", [SLICE_PAD, D], f32, kind="Internal")
             for k in range(3)]
    h_tab = [nc.dram_tensor(f"htab{k}", [2 * HALF, D], f32, kind="Internal",
                            addr_space="Shared") for k in range(3)]

    SLAB = NBLK * D

    with tile.TileContext(nc) as tc:
        with (
            tc.tile_pool(name="const", bufs=1) as constp,
            tc.tile_pool(name="stg", bufs=2) as stgp,
        ):
            idx_sb = constp.tile([128, ncalls * G * 8], mybir.dt.int16, name="idx_sb")
            w_sb = constp.tile([128, TOT], f32, name="w_sb")
            scat_sb = constp.tile([128, SLICE_PAD // 16], mybir.dt.int16,
                                  name="scat_sb")
            for gi in range(8):
                nc.sync.dma_start(idx_sb[16 * gi:16 * (gi + 1), :], idx_in.ap())
            nc.sync.dma_start(w_sb[:], w_in.ap())
            for gi in range(8):
                nc.sync.dma_start(scat_sb[16 * gi:16 * (gi + 1), :], scat_in.ap())

            stag0 = constp.tile([128, SLAB], f32, name="stag0")
            stag1 = constp.tile([128, SLAB], f32, name="stag1")
            stag = [stag0, stag1]
            hi_sb = constp.tile([128, SLAB], f32, name="hi_sb")
            Tm1 = constp.tile([128, SLAB], f32, name="Tm1")
            Tm2 = constp.tile([128, SLAB], f32, name="Tm2")
            Tcur = constp.tile([128, SLAB], f32, name="Tcur")

            def sl_dram(t, k=None):
                ap = t.ap() if k is None else t.ap()[k, :, :]
                return ap.rearrange("(b p) d -> p b d", p=128)

            def sb3(t):
                return t[:].rearrange("p (b d) -> p b d", d=D)

            nc.sync.dma_start(sb3(Tm1), sl_dram(x_sl))
            nc.gpsimd.memset(hi_sb[:], 0.0)
            for k in range(4):
                nc.sync.dma_start(sl_dram(bounce[k]), sb3(hi_sb))
            nc.sync.dma_start(x_bounce.ap(), x_sl.ap())
            if timing_rep:
                for gi in range(8):
                    nc.sync.dma_start(
                        x_glob.ap()[gi * SLICE_PAD:(gi + 1) * SLICE_PAD, :],
                        x_sl.ap())
            else:
                nc.gpsimd.collective_compute(
                    "AllGather", mybir.AluOpType.bypass,
                    replica_groups=[list(range(NCORES))],
                    ins=[x_bounce.ap()], outs=[x_glob.ap()])

            rep_cm = tc.For_i(0, timing_rep, 1) if timing_rep else None
            if rep_cm is not None:
                rep_cm.__enter__()
            for k in range(4):
                src_tab = (x_glob.ap() if (k == 0 or timing_rep)
                           else h_tab[k - 1].ap())
                views = [src_tab[0:HALF, :], src_tab[HALF:2 * HALF, :]]
                for (p, base, n_pad, blist) in groups:
                    sgt = stgp.tile([128, GCH, D], f32, tag="sg",
                                    name=f"sg_{k}_{p}_{base}")
                    for j in range(n_pad // G):
                        cstart = base + j * G
                        nc.gpsimd.dma_gather(
                            out_ap=sgt[:, j * G:(j + 1) * G, :],
                            in_ap=views[p],
                            idxs_ap=idx_sb[:, cstart * 8:(cstart + G) * 8],
                            num_idxs=G * 128,
                            num_idxs_reg=G * 128,
                            elem_size=D,
                            single_packet=False,
                            queue_num=(cstart // G) % 2,
                        )
                    # scale by w: staging *= w (broadcast along feature dim)
                    wv = w_sb[:, base:base + n_pad, None].to_broadcast(
                        [128, n_pad, D])
                    nc.vector.tensor_tensor(
                        out=sgt[:, 0:n_pad, :], in0=sgt[:, 0:n_pad, :],
                        in1=wv, op=mybir.AluOpType.mult)
                    # per-block rank reduction
                    for (b, off) in blist:
                        rb = int(R[p, b])
                        inap = sgt[:, off:off + rb, :].rearrange(
                            "p r d -> p d r")
                        nc.vector.tensor_reduce(
                            out=stag[p][:, b * D:(b + 1) * D],
                            in_=inap, axis=mybir.AxisListType.X,
                            op=mybir.AluOpType.add)

                nc.gpsimd.dma_scatter_add(
                    out_ap=bounce[k].ap(),
                    in_ap=sb3(stag[1]),
                    idxs_ap=scat_sb[:],
                    num_idxs=SLICE_PAD,
                    num_idxs_reg=SLICE_PAD,
                    elem_size=D,
                    single_packet=False,
                )
                nc.sync.dma_start(sb3(hi_sb), sl_dram(bounce[k]))
                nc.vector.tensor_tensor(out=hi_sb[:], in0=stag[0][:],
                                        in1=hi_sb[:], op=mybir.AluOpType.add)
                if k == 0:
                    nc.vector.tensor_tensor(out=Tcur[:], in0=Tm1[:],
                                            in1=hi_sb[:],
                                            op=mybir.AluOpType.subtract)
                else:
                    nc.vector.tensor_tensor(out=hi_sb[:], in0=Tm1[:],
                                            in1=hi_sb[:],
                                            op=mybir.AluOpType.subtract)
                    nc.vector.scalar_tensor_tensor(
                        out=Tcur[:], in0=hi_sb[:], scalar=2.0, in1=Tm2[:],
                        op0=mybir.AluOpType.mult,
                        op1=mybir.AluOpType.subtract)
                nc.sync.dma_start(sl_dram(out_t, k), sb3(Tcur))
                if k < 3:
                    nc.sync.dma_start(sl_dram(ag_in[k]), sb3(Tcur))
                    if not timing_rep:
                        nc.gpsimd.collective_compute(
                            "AllGather",
                            mybir.AluOpType.bypass,
                            replica_groups=[list(range(NCORES))],
                            ins=[ag_in[k].ap()],
                            outs=[h_tab[k].ap()],
                        )
                    nc.vector.tensor_copy(Tm2[:], Tm1[:])
                    nc.vector.tensor_copy(Tm1[:], Tcur[:])
            if rep_cm is not None:
                rep_cm.__exit__(None, None, None)

    nc.compile()
    return nc


def kernel(x, edge_index, edge_weight):
    from concourse.bass_utils import run_bass_kernel_spmd

    x = np.asarray(x, dtype=np.float32)
    import hashlib
    ei_b = np.ascontiguousarray(edge_index)
    ew_b = np.ascontiguousarray(edge_weight)
    key = hashlib.md5(ei_b.tobytes() + ew_b.tobytes()).hexdigest()
    if _cache.get("key") != key:
        _cache.clear()
        _cache["key"] = key
        ei = np.asarray(edge_index)
        ew = np.asarray(edge_weight, dtype=np.float32)
        meta, idx_w, w_all, scat_w = _build_structure(ei, ew)
        nc = _build_program(meta)
        _cache["built"] = (meta, idx_w, w_all, scat_w, nc)
    meta, idx_w, w_all, scat_w, nc = _cache["built"]

    gid = meta["gid"]
    x_glob = np.zeros((2 * HALF, D), np.float32)
    x_glob[gid] = x
    in_maps = []
    for c in range(NCORES):
        in_maps.append({
            "x_sl": x_glob[c * SLICE_PAD:(c + 1) * SLICE_PAD],
            "idx": idx_w[c],
            "w": w_all[c],
            "scat": scat_w[c],
        })
    res = run_bass_kernel_spmd(nc, in_maps, core_ids=list(range(NCORES)))

    out = np.empty((5, N, D), np.float32)
    out[0] = x
    node_core = meta["node_core"]
    rank_lo = meta["rank"][:, 0, :]
    for c in range(NCORES):
        o = res.results[c]["out"]
        nodes = np.nonzero(node_core == c)[0]
        rk = rank_lo[c, nodes - c * SLICE]
        out[1:, nodes, :] = o[:, rk, :]
    return out


# revision 10
# speedup vs baseline: 1.1927x; 1.1927x over previous
"""DiffusionStep (Chebyshev K=4) GNN message passing on 8 Trainium2 cores.

Sharding: 1D dst-shard (6250 dst/core, padded 6272) with a src-half two-pass
per propagate so dma_gather's int16 indices stay < 25088. Aggregation is
rank-major: blocks of 128 dst (sorted by per-pass in-degree) are packed into
"groups" (<= 64 chunks of 128 slots). Per group: dma_gather calls write
h[src] rows straight into a staging slab, one tensor_tensor multiply applies
edge weights (step-0 broadcast of w), and one strided tensor_reduce per block
sums ranks. The hi-pass accumulator (its own degree order) is re-permuted via
dma_scatter_add through a zero-donated DRAM bounce. Chebyshev runs on SBUF
slabs; T_k slices are AllGathered into a shared DRAM table for the next
propagate. All permutations are composed into static host-built tables.
"""

import numpy as np

N = 50000
D = 64
NCORES = 8
SLICE = 6250
SLICE_PAD = 6272            # 49 * 128
NBLK = SLICE_PAD // 128     # 49
HALF = 25088                # rows per gather table (= 4 * 6272)
G = 16                      # chunks per dma_gather call (2048 idx)
GCH = 128                   # staging capacity in chunks per group

_cache = {}


def _build_structure(edge_index, edge_weight):
    src = edge_index[0].astype(np.int64)
    dst = edge_index[1].astype(np.int64)
    w = edge_weight.astype(np.float32)

    node_core = np.minimum(np.arange(N) // SLICE, NCORES - 1)
    node_local = np.arange(N) - node_core * SLICE
    src_half = (np.minimum(src // SLICE, NCORES - 1) >= 4).astype(np.int64)
    dcore = np.minimum(dst // SLICE, NCORES - 1)

    deg = np.zeros((NCORES, 2, SLICE_PAD), np.int64)
    for c in range(NCORES):
        m = dcore == c
        dl = dst[m] - c * SLICE
        s = src_half[m]
        for p in range(2):
            deg[c, p] = np.bincount(dl[s == p], minlength=SLICE_PAD)

    pi = np.zeros((NCORES, 2, SLICE_PAD), np.int64)
    rank = np.zeros((NCORES, 2, SLICE_PAD), np.int64)
    for c in range(NCORES):
        for p in range(2):
            order = np.argsort(deg[c, p], kind="stable")
            pi[c, p] = order
            rank[c, p, order] = np.arange(SLICE_PAD)

    R = np.zeros((2, NBLK), np.int64)
    for p in range(2):
        for b in range(NBLK):
            mx = 1
            for c in range(NCORES):
                dloc = pi[c, p, b * 128:(b + 1) * 128]
                mx = max(mx, int(deg[c, p, dloc].max()))
            R[p, b] = mx

    gid = node_core * SLICE_PAD + rank[node_core, 0, node_local]

    # group packing per pass: consecutive blocks, sum R <= GCH, padded to G
    groups = []      # list of (pass, chunk_base, nch_padded, [(b, off_in_group)])
    chunk_base = 0
    blk_chunk = np.zeros((2, NBLK), np.int64)   # global chunk pos of block start
    for p in range(2):
        cur = []
        cur_n = 0
        for b in range(NBLK):
            rb = int(R[p, b])
            if cur and cur_n + rb > GCH:
                n_pad = (cur_n + G - 1) // G * G
                groups.append((p, chunk_base, n_pad, cur))
                chunk_base += n_pad
                cur, cur_n = [], 0
            cur.append((b, cur_n))
            blk_chunk[p, b] = chunk_base + cur_n   # provisional (fixed below)
            cur_n += rb
        if cur:
            n_pad = (cur_n + G - 1) // G * G
            groups.append((p, chunk_base, n_pad, cur))
            chunk_base += n_pad
    TOT = chunk_base
    # fix blk_chunk: recompute with final bases
    for (p, base, n_pad, blist) in groups:
        for (b, off) in blist:
            blk_chunk[p, b] = base + off

    idx_all = np.zeros((NCORES, TOT * 128), np.int64)
    w_all = np.zeros((NCORES, 128, TOT), np.float32)

    for c in range(NCORES):
        e_idx = np.nonzero(dcore == c)[0]
        s = src_half[e_idx]
        for p in range(2):
            sel = e_idx[s == p]
            d_loc = dst[sel] - c * SLICE
            r_of_dst = rank[c, p, d_loc]
            order = np.argsort(r_of_dst, kind="stable")
            sel = sel[order]
            d_rank = r_of_dst[order]
            counts = np.bincount(d_rank, minlength=SLICE_PAD)
            starts = np.concatenate([[0], np.cumsum(counts)[:-1]])
            within = np.arange(len(sel)) - starts[d_rank]
            b_of = d_rank // 128
            chunk = blk_chunk[p, b_of] + within
            slot = chunk * 128 + (d_rank % 128)
            g = gid[src[sel]]
            idx_all[c, slot] = g - (0 if p == 0 else HALF)
            w_all[c, slot % 128, slot // 128] = w[sel]

    ncalls = TOT // G
    idx_wrapped = np.zeros((NCORES, 16, ncalls * G * 8), np.int16)
    for c in range(NCORES):
        a = idx_all[c].reshape(ncalls, G * 128)
        for call in range(ncalls):
            wrp = a[call].reshape(-1, 16).T.astype(np.uint16).view(np.int16)
            idx_wrapped[c, :, call * G * 8:(call + 1) * G * 8] = wrp

    scat_wrapped = np.zeros((NCORES, 16, SLICE_PAD // 16), np.int16)
    for c in range(NCORES):
        sc = rank[c, 0, pi[c, 1]]
        scat_wrapped[c] = sc.reshape(-1, 16).T.astype(np.uint16).view(np.int16)

    meta = dict(R=R, TOT=TOT, ncalls=ncalls, groups=groups, pi=pi,
                rank=rank, gid=gid, node_core=node_core)
    return meta, idx_wrapped, w_all, scat_wrapped


def _build_program(meta, timing_rep=0):
    import concourse.bacc as bacc
    import concourse.mybir as mybir
    import concourse.tile as tile

    R = meta["R"]
    TOT = meta["TOT"]
    ncalls = meta["ncalls"]
    groups = meta["groups"]

    nc = bacc.Bacc(num_devices=NCORES)
    f32 = mybir.dt.float32

    x_sl = nc.dram_tensor("x_sl", [SLICE_PAD, D], f32, kind="ExternalInput")
    x_bounce = nc.dram_tensor("x_bounce", [SLICE_PAD, D], f32, kind="Internal")
    x_glob = nc.dram_tensor("x_glob", [2 * HALF, D], f32, kind="Internal",
                            addr_space="Shared")
    idx_in = nc.dram_tensor("idx", [16, ncalls * G * 8], mybir.dt.int16,
                            kind="ExternalInput")
    w_in = nc.dram_tensor("w", [128, TOT], f32, kind="ExternalInput")
    scat_in = nc.dram_tensor("scat", [16, SLICE_PAD // 16], mybir.dt.int16,
                             kind="ExternalInput")
    out_t = nc.dram_tensor("out", [4, SLICE_PAD, D], f32, kind="ExternalOutput")
    bounce = [nc.dram_tensor(f"bnc{k}", [SLICE_PAD, D], f32,
                             kind="Internal") for k in range(4)]
    ag_in = [nc.dram_tensor(f"agin{k}", [SLICE_PAD, D], f32, kind="Internal")
             for k in range(3)]
    h_tab = [nc.dram_tensor(f"htab{k}", [2 * HALF, D], f32, kind="Internal",
                            addr_space="Shared") for k in range(3)]

    SLAB = NBLK * D

    with tile.TileContext(nc) as tc:
        with (
            tc.tile_pool(name="const", bufs=1) as constp,
            tc.tile_pool(name="stg", bufs=2) as stgp,
        ):
            idx_sb = constp.tile([128, ncalls * G * 8], mybir.dt.int16, name="idx_sb")
            w_sb = constp.tile([128, TOT], f32, name="w_sb")
            scat_sb = constp.tile([128, SLICE_PAD // 16], mybir.dt.int16,
                                  name="scat_sb")
            for gi in range(8):
                nc.sync.dma_start(idx_sb[16 * gi:16 * (gi + 1), :], idx_in.ap())
            nc.sync.dma_start(w_sb[:], w_in.ap())
            for gi in range(8):
                nc.sync.dma_start(scat_sb[16 * gi:16 * (gi + 1), :], scat_in.ap())

            stag0 = constp.tile([128, SLAB], f32, name="stag0")
            stag1 = constp.tile([128, SLAB], f32, name="stag1")
            stag = [stag0, stag1]
            hi_sb = constp.tile([128, SLAB], f32, name="hi_sb")
            Tm1 = constp.tile([128, SLAB], f32, name="Tm1")
            Tm2 = constp.tile([128, SLAB], f32, name="Tm2")
            Tcur = constp.tile([128, SLAB], f32, name="Tcur")

            def sl_dram(t, k=None):
                ap = t.ap() if k is None else t.ap()[k, :, :]
                return ap.rearrange("(b p) d -> p b d", p=128)

            def sb3(t):
                return t[:].rearrange("p (b d) -> p b d", d=D)

            nc.sync.dma_start(sb3(Tm1), sl_dram(x_sl))
            nc.gpsimd.memset(hi_sb[:], 0.0)
            for k in range(4):
                nc.sync.dma_start(sl_dram(bounce[k]), sb3(hi_sb))
            nc.sync.dma_start(x_bounce.ap(), x_sl.ap())
            if timing_rep:
                for gi in range(8):
                    nc.sync.dma_start(
                        x_glob.ap()[gi * SLICE_PAD:(gi + 1) * SLICE_PAD, :],
                        x_sl.ap())
            else:
                nc.gpsimd.collective_compute(
                    "AllGather", mybir.AluOpType.bypass,
                    replica_groups=[list(range(NCORES))],
                    ins=[x_bounce.ap()], outs=[x_glob.ap()])

            rep_cm = tc.For_i(0, timing_rep, 1) if timing_rep else None
            if rep_cm is not None:
                rep_cm.__enter__()
            for k in range(4):
                src_tab = (x_glob.ap() if (k == 0 or timing_rep)
                           else h_tab[k - 1].ap())
                views = [src_tab[0:HALF, :], src_tab[HALF:2 * HALF, :]]
                for (p, base, n_pad, blist) in groups:
                    sgt = stgp.tile([128, GCH, D], f32, tag="sg",
                                    name=f"sg_{k}_{p}_{base}")
                    for j in range(n_pad // G):
                        cstart = base + j * G
                        nc.gpsimd.dma_gather(
                            out_ap=sgt[:, j * G:(j + 1) * G, :],
                            in_ap=views[p],
                            idxs_ap=idx_sb[:, cstart * 8:(cstart + G) * 8],
                            num_idxs=G * 128,
                            num_idxs_reg=G * 128,
                            elem_size=D,
                            single_packet=False,
                        )
                    # scale by w: staging *= w (broadcast along feature dim)
                    wv = w_sb[:, base:base + n_pad, None].to_broadcast(
                        [128, n_pad, D])
                    nc.vector.tensor_tensor(
                        out=sgt[:, 0:n_pad, :], in0=sgt[:, 0:n_pad, :],
                        in1=wv, op=mybir.AluOpType.mult)
                    # per-block rank reduction
                    for (b, off) in blist:
                        rb = int(R[p, b])
                        inap = sgt[:, off:off + rb, :].rearrange(
                            "p r d -> p d r")
                        nc.vector.tensor_reduce(
                            out=stag[p][:, b * D:(b + 1) * D],
                            in_=inap, axis=mybir.AxisListType.X,
                            op=mybir.AluOpType.add)

                nc.gpsimd.dma_scatter_add(
                    out_ap=bounce[k].ap(),
                    in_ap=sb3(stag[1]),
                    idxs_ap=scat_sb[:],
                    num_idxs=SLICE_PAD,
                    num_idxs_reg=SLICE_PAD,
                    elem_size=D,
                    single_packet=False,
                )
                nc.sync.dma_start(sb3(hi_sb), sl_dram(bounce[k]))
                nc.vector.tensor_tensor(out=hi_sb[:], in0=stag[0][:],
                                        in1=hi_sb[:], op=mybir.AluOpType.add)
                if k == 0:
                    nc.vector.tensor_tensor(out=Tcur[:], in0=Tm1[:],
                                            in1=hi_sb[:],
                                            op=mybir.AluOpType.subtract)
                else:
                    nc.vector.tensor_tensor(out=hi_sb[:], in0=Tm1[:],
                                            in1=hi_sb[:],
                                            op=mybir.AluOpType.subtract)
                    nc.vector.scalar_tensor_tensor(
                        out=Tcur[:], in0=hi_sb[:], scalar=2.0, in1=Tm2[:],
                        op0=mybir.AluOpType.mult,
                        op1=mybir.AluOpType.subtract)
                nc.sync.dma_start(sl_dram(out_t, k), sb3(Tcur))
                if k < 3:
                    nc.sync.dma_start(sl_dram(ag_in[k]), sb3(Tcur))
                    if not timing_rep:
                        nc.gpsimd.collective_compute(
                            "AllGather",
                            mybir.AluOpType.bypass,
                            replica_groups=[list(range(NCORES))],
                            ins=[ag_in[k].ap()],
                            outs=[h_tab[k].ap()],
                        )
                    nc.vector.tensor_copy(Tm2[:], Tm1[:])
                    nc.vector.tensor_copy(Tm1[:], Tcur[:])
            if rep_cm is not None:
                rep_cm.__exit__(None, None, None)

    nc.compile()
    return nc


def kernel(x, edge_index, edge_weight):
    from concourse.bass_utils import run_bass_kernel_spmd

    x = np.asarray(x, dtype=np.float32)
    import hashlib
    ei_b = np.ascontiguousarray(edge_index)
    ew_b = np.ascontiguousarray(edge_weight)
    key = hashlib.md5(ei_b.tobytes() + ew_b.tobytes()).hexdigest()
    if _cache.get("key") != key:
        _cache.clear()
        _cache["key"] = key
        ei = np.asarray(edge_index)
        ew = np.asarray(edge_weight, dtype=np.float32)
        meta, idx_w, w_all, scat_w = _build_structure(ei, ew)
        nc = _build_program(meta)
        _cache["built"] = (meta, idx_w, w_all, scat_w, nc)
    meta, idx_w, w_all, scat_w, nc = _cache["built"]

    gid = meta["gid"]
    x_glob = np.zeros((2 * HALF, D), np.float32)
    x_glob[gid] = x
    in_maps = []
    for c in range(NCORES):
        in_maps.append({
            "x_sl": x_glob[c * SLICE_PAD:(c + 1) * SLICE_PAD],
            "idx": idx_w[c],
            "w": w_all[c],
            "scat": scat_w[c],
        })
    res = run_bass_kernel_spmd(nc, in_maps, core_ids=list(range(NCORES)))

    out = np.empty((5, N, D), np.float32)
    out[0] = x
    node_core = meta["node_core"]
    rank_lo = meta["rank"][:, 0, :]
    for c in range(NCORES):
        o = res.results[c]["out"]
        nodes = np.nonzero(node_core == c)[0]
        rk = rank_lo[c, nodes - c * SLICE]
        out[1:, nodes, :] = o[:, rk, :]
    return out
